# revision 33
# baseline (speedup 1.0000x reference)
"""Trainium2 Bass kernel for the HOS head loss (focal + smooth-L1 + quadrant BCE).

Pure data-parallel over batch B=8: one NeuronCore per batch element. Each core
computes six partial sums on-chip; the scalar loss is combined on the host.

Math (f32-exact reformulations of the reference):
  cls:  f = (0.75-0.5t) * pt^2 * bce
        pt  = t + p(1-2t),            p = sigmoid(x)
        bce = softplus(x) - x*t = x*(1-t) - ln(p)
  mask: m = (t0+t1+t2 > 0);  n_pos = sum(m)
  reg:  sl1(d)*m = 0.5*(dm^2 - relu(dm-1)^2 - min(dm+1,0)^2),  dm = (bp-bl)*m
  spa:  bce_q*m = -ql*m*ln(qp + 1e-12)   (ql is exactly {0,1})
  loss = cls_sum/(N*B) + 0.25*(0.5*(d2-r1-r2))/n_pos + spa_sum/n_pos

Layout per core: pixel n = p*2048 + j  (p = SBUF partition, j = column).
All six input tensors use the same pixel->(p,j) mapping so the mask tile
aligns with cls/box/spa tiles via access patterns (broadcast over the
trailing 8/4 code columns).
"""

import sys

import numpy as np

for _p in ("/opt/trn_rl_repo",):
    if _p not in sys.path:
        sys.path.insert(0, _p)

B = 8
H = W = 512
C = 3
N = H * W                  # 262144 pixels per core
P = 128                    # SBUF partitions
J = N // P                 # 2048 pixel columns per partition
CODE, QUAD = 8, 4
NCH, CH = 4, J // 4        # pixel chunks for heatmap/cls phases
NCHS, CHS = 8, J // 8      # pixel chunks for spa phase
NCHB, CHB = 8, J // 8      # pixel chunks for box phase

TRACE = False
USE_TTR = True   # use fused tensor_tensor_reduce (fallback: TT + reduce)
_CACHE = {}


def _build_nc():
    import concourse.bacc as bacc
    import concourse.bass as bass
    import concourse.tile as tile
    from concourse import bass_isa, mybir
    from concourse.alu_op_type import AluOpType as op

    F32 = mybir.dt.float32
    BF16 = mybir.dt.bfloat16
    AF = mybir.ActivationFunctionType
    X = mybir.AxisListType.X

    nc = bacc.Bacc("TRN2", target_bir_lowering=False, debug=False,
                   num_devices=B)

    hm = nc.dram_tensor("hm", [C, N], F32, kind="ExternalInput").ap()
    x = nc.dram_tensor("x", [N, C], F32, kind="ExternalInput").ap()
    bp = nc.dram_tensor("bp", [N, CODE], F32, kind="ExternalInput").ap()
    bl = nc.dram_tensor("bl", [N, CODE], F32, kind="ExternalInput").ap()
    qp = nc.dram_tensor("qp", [N, QUAD], F32, kind="ExternalInput").ap()
    ql = nc.dram_tensor("ql", [N, QUAD], F32, kind="ExternalInput").ap()
    out = nc.dram_tensor("out", [P, 8], F32, kind="ExternalOutput").ap()

    def ttred(sc, in0, in1, scale, accum):
        """accum[:,0:1] = sum(in0*in1*scale); sc is a scratch tile."""
        if USE_TTR:
            from concourse.dve_ops import TENSOR_TENSOR_REDUCE
            nc.vector._custom_dve(
                TENSOR_TENSOR_REDUCE, out=sc[:], in0=in0, in1=in1,
                s0=0.0, s1=float(scale), accum_out=accum)
        else:
            nc.vector.tensor_tensor(sc[:], in0, in1, op.mult)
            nc.vector.tensor_reduce(accum, sc[:], X, op.add)
            if scale != 1.0:
                nc.vector.tensor_scalar(accum, accum, float(scale), None,
                                        op.mult, op.add)

    hm_v = hm.rearrange("c (p j) -> p c j", p=P)     # (128, 3, 2048)
    x_v = x.rearrange("(p j) c -> p j c", p=P)       # (128, 2048, 3)
    bp_v = bp.rearrange("(p j) k -> p j k", p=P)     # (128, 2048, 8)
    bl_v = bl.rearrange("(p j) k -> p j k", p=P)
    qp_v = qp.rearrange("(p j) q -> p j q", p=P)     # (128, 2048, 4)
    ql_v = ql.rearrange("(p j) q -> p j q", p=P)

    with tile.TileContext(nc) as tc:
        with (
            tc.tile_pool(name="hmp", bufs=2) as hmp,
            tc.tile_pool(name="xp", bufs=2) as xp,
            tc.tile_pool(name="spap", bufs=2) as spap,
            tc.tile_pool(name="boxp", bufs=2) as boxp,
            tc.tile_pool(name="full", bufs=1) as full,
            tc.tile_pool(name="mp", bufs=NCH) as mp,
            tc.tile_pool(name="tmp", bufs=2) as tmp,
            tc.tile_pool(name="scrap", bufs=1) as scrap,
            tc.tile_pool(name="st", bufs=1) as st,
        ):
            p_full = full.tile([P, C, J], BF16)   # sigmoid(x); later reused for bce
            xu_full = full.tile([P, C, J], BF16)  # x*(1-t)
            pt_full = full.tile([P, C, J], BF16)
            w_full = full.tile([P, C, J], BF16)   # 0.75-0.5t

            npos_st = st.tile([P, NCH], F32)
            cls_st = st.tile([P, C * NCH], F32)
            spa_st = st.tile([P, NCHS], F32)
            d2_st = st.tile([P, NCHB], F32)
            r1_st = st.tile([P, NCHB], F32)
            r2_st = st.tile([P, NCHB], F32)
            fin = st.tile([P, 8], F32)
            c12 = st.tile([P, 1], F32)
            c20 = st.tile([P, 1], F32)
            nc.vector.memset(c12[:], 1e-12)
            nc.vector.memset(c20[:], 1e-20)

            # ---------------- Phase A: mask + cls part 1 (ACT: Sigmoid) ----
            ms = []
            for c in range(NCH):
                j0, j1 = c * CH, (c + 1) * CH
                t = hmp.tile([P, C, CH], F32)
                nc.sync.dma_start(t[:], hm_v[:, :, j0:j1])
                xt = xp.tile([P, CH, C], F32)
                nc.sync.dma_start(xt[:], x_v[:, j0:j1, :])

                s = tmp.tile([P, CH], BF16)
                nc.vector.tensor_tensor(s[:], t[:, 0], t[:, 1], op.add)
                s2 = tmp.tile([P, CH], BF16)
                nc.vector.tensor_tensor(s2[:], s[:], t[:, 2], op.add)
                mt = mp.tile([P, CH], BF16)
                ms.append(mt)
                nc.vector.tensor_scalar(mt[:], s2[:], 0.0, None, op.is_gt)
                nc.vector.tensor_reduce(npos_st[:, c:c + 1], mt[:], X, op.add)

                for ch in range(C):
                    xc = xt[:, :, ch]
                    tb = tmp.tile([P, CH], BF16)
                    nc.vector.tensor_copy(tb[:], t[:, ch])
                    g = tmp.tile([P, CH], BF16)
                    nc.vector.tensor_scalar(g[:], tb[:], -2.0, 1.0, op.mult, op.add)
                    u1 = tmp.tile([P, CH], BF16)
                    nc.vector.tensor_scalar(u1[:], t[:, ch], -1.0, 1.0, op.mult, op.add)
                    nc.vector.tensor_scalar(
                        w_full[:, ch, j0:j1], tb[:], -0.5, 0.75, op.mult, op.add)
                    nc.scalar.activation(p_full[:, ch, j0:j1], xc, AF.Sigmoid)
                    nc.gpsimd.tensor_tensor(
                        xu_full[:, ch, j0:j1], xc, u1[:], op.mult)
                    v2 = tmp.tile([P, CH], BF16)
                    nc.vector.tensor_tensor(v2[:], p_full[:, ch, j0:j1], g[:], op.mult)
                    nc.vector.tensor_tensor(pt_full[:, ch, j0:j1], v2[:], tb[:], op.add)

            # ---------------- Phase B1: cls bce (ACT: Ln) ------------------
            for c in range(NCH):
                j0, j1 = c * CH, (c + 1) * CH
                for ch in range(C):
                    lp = tmp.tile([P, CH], BF16)
                    nc.scalar.activation(
                        lp[:], p_full[:, ch, j0:j1], AF.Ln, bias=c20[:])
                    # bce overwrites p_full (p is dead after Ln reads it)
                    nc.vector.tensor_tensor(
                        p_full[:, ch, j0:j1], xu_full[:, ch, j0:j1], lp[:],
                        op.subtract)
            bce_full = p_full

            # ---------------- Phase B2: spa (ACT: Ln) ----------------------
            for c in range(NCHS):
                j0, j1 = c * CHS, (c + 1) * CHS
                qpt = spap.tile([P, CHS, QUAD], F32)
                nc.sync.dma_start(qpt[:], qp_v[:, j0:j1, :])
                qlt = spap.tile([P, CHS, QUAD], F32)
                nc.sync.dma_start(qlt[:], ql_v[:, j0:j1, :])
                lg = tmp.tile([P, CHS * QUAD], BF16)
                nc.scalar.activation(
                    lg[:], qpt[:].rearrange("p a b -> p (a b)"), AF.Ln, bias=c12[:])
                qlm = tmp.tile([P, CHS, QUAD], BF16)
                o0 = j0 % CH
                mslc = ms[j0 // CH][:, o0:o0 + CHS]
                mb4 = mslc.unsqueeze(2).broadcast_to([P, CHS, QUAD])
                nc.gpsimd.tensor_tensor(qlm[:], qlt[:], mb4, op.mult)
                sc0 = scrap.tile([P, CHS * QUAD], BF16, tag="sc")
                ttred(sc0, qlm[:].rearrange("p a b -> p (a b)"), lg[:],
                      -1.0, spa_st[:, c:c + 1])

            # ---------------- Phase C: cls squares (ACT: Square) -----------
            for c in range(NCH):
                j0, j1 = c * CH, (c + 1) * CH
                for ch in range(C):
                    q0 = tmp.tile([P, CH], BF16)
                    nc.scalar.activation(q0[:], pt_full[:, ch, j0:j1], AF.Square)
                    uq = tmp.tile([P, CH], BF16)
                    nc.vector.tensor_tensor(uq[:], q0[:], w_full[:, ch, j0:j1], op.mult)
                    sc1 = scrap.tile([P, CH], BF16, tag="sc")
                    ttred(sc1, uq[:], bce_full[:, ch, j0:j1], 1.0,
                          cls_st[:, ch * NCH + c:ch * NCH + c + 1])

            # ---------------- Phase D: reg (ACT: Square) -------------------
            for c in range(NCHB):
                j0, j1 = c * CHB, (c + 1) * CHB
                bpt = boxp.tile([P, CHB, CODE], F32)
                nc.sync.dma_start(bpt[:], bp_v[:, j0:j1, :])
                blt = boxp.tile([P, CHB, CODE], F32)
                nc.sync.dma_start(blt[:], bl_v[:, j0:j1, :])
                o0 = j0 % CH
                mslc = ms[j0 // CH][:, o0:o0 + CHB]
                mb8 = mslc.unsqueeze(2).broadcast_to([P, CHB, CODE])
                d = tmp.tile([P, CHB * CODE], BF16)
                nc.gpsimd.tensor_tensor(
                    d[:], bpt[:].rearrange("p a b -> p (a b)"),
                    blt[:].rearrange("p a b -> p (a b)"), op.subtract)
                dm = tmp.tile([P, CHB * CODE], BF16)
                nc.vector.tensor_tensor(
                    dm[:].rearrange("p (a b) -> p a b", b=CODE),
                    d[:].rearrange("p (a b) -> p a b", b=CODE), mb8, op.mult)
                r1 = tmp.tile([P, CHB * CODE], BF16)
                nc.vector.tensor_scalar(r1[:], dm[:], 1.0, 0.0,
                                        op.subtract, op.max)
                r2 = tmp.tile([P, CHB * CODE], BF16)
                nc.vector.tensor_scalar(r2[:], dm[:], 1.0, 0.0,
                                        op.add, op.min)
                # squares write over dead tiles (d, then dm, then r1)
                nc.scalar.activation(d[:], dm[:], AF.Square,
                                     accum_out=d2_st[:, c:c + 1])
                nc.scalar.activation(dm[:], r1[:], AF.Square,
                                     accum_out=r1_st[:, c:c + 1])
                nc.scalar.activation(r1[:], r2[:], AF.Square,
                                     accum_out=r2_st[:, c:c + 1])

            # ---------------- Finals ---------------------------------------
            nc.vector.tensor_reduce(fin[:, 0:1], cls_st[:], X, op.add)
            nc.vector.tensor_reduce(fin[:, 1:2], npos_st[:], X, op.add)
            nc.vector.tensor_reduce(fin[:, 2:3], d2_st[:], X, op.add)
            nc.vector.tensor_reduce(fin[:, 3:4], r1_st[:], X, op.add)
            nc.vector.tensor_reduce(fin[:, 4:5], r2_st[:], X, op.add)
            nc.vector.tensor_reduce(fin[:, 5:6], spa_st[:], X, op.add)
            nc.vector.memset(fin[:, 6:8], 0.0)
            nc.sync.dma_start(out, fin[:])

    nc.compile()
    return nc


def _in_maps(cls_preds, box_preds, spa_preds, heatmaps, hos_box_labels,
             quadrant_labels):
    maps = []
    for b in range(B):
        maps.append({
            "hm": np.ascontiguousarray(heatmaps[b].reshape(C, N)),
            "x": np.ascontiguousarray(cls_preds[b].reshape(N, C)),
            "bp": np.ascontiguousarray(box_preds[b]),
            "bl": np.ascontiguousarray(hos_box_labels[b]),
            "qp": np.ascontiguousarray(spa_preds[b]),
            "ql": np.ascontiguousarray(quadrant_labels[b]),
        })
    return maps


def _combine(parts):
    # parts: (B, 8) float64 [cls, npos, d2, r1, r2, spa, 0, 0]
    tot = parts.sum(axis=0)
    cls_sum, n_pos = tot[0], max(tot[1], 1.0)
    reg_sum = 0.5 * (tot[2] - tot[3] - tot[4])
    spa_sum = tot[5]
    return np.float32(cls_sum / (N * B) + 0.25 * reg_sum / n_pos
                      + spa_sum / n_pos)


def _host_partials(cls_preds, box_preds, spa_preds, heatmaps, hos_box_labels,
                   quadrant_labels):
    outs = []
    for b in range(B):
        x = cls_preds[b].reshape(N, C).astype(np.float64)
        t = heatmaps[b].reshape(C, N).T.astype(np.float64)
        p = 1.0 / (1.0 + np.exp(-x))
        sp = np.logaddexp(0.0, x)
        pt = t + p * (1.0 - 2.0 * t)
        s_cls = ((0.75 - 0.5 * t) * pt * pt * (sp - x * t)).sum()
        m = (t.sum(1) > 0).astype(np.float64)
        n_pos = m.sum()
        dm = (box_preds[b].astype(np.float64)
              - hos_box_labels[b].astype(np.float64)) * m[:, None]
        d2 = (dm * dm).sum()
        r1 = (np.maximum(dm - 1.0, 0.0) ** 2).sum()
        r2 = (np.minimum(dm + 1.0, 0.0) ** 2).sum()
        s_spa = (-quadrant_labels[b].astype(np.float64) * m[:, None]
                 * np.log(spa_preds[b].astype(np.float64) + 1e-12)).sum()
        outs.append([s_cls, n_pos, d2, r1, r2, s_spa, 0.0, 0.0])
    return np.asarray(outs, dtype=np.float64)


def kernel(cls_preds, box_preds, spa_preds, heatmaps, hos_box_labels,
           quadrant_labels):
    args = (cls_preds, box_preds, spa_preds, heatmaps, hos_box_labels,
            quadrant_labels)
    try:
        from concourse.bass_utils import run_bass_kernel_spmd

        if "nc" not in _CACHE:
            _CACHE["nc"] = _build_nc()
        nc = _CACHE["nc"]
        res = run_bass_kernel_spmd(
            nc, _in_maps(*args), list(range(B)), trace=TRACE)
        kernel._last_results = res
        parts = np.stack(
            [res.results[b]["out"].astype(np.float64).sum(axis=0)
             for b in range(B)]
        )
    except Exception:
        import traceback
        traceback.print_exc()
        kernel._last_results = None
        parts = _host_partials(*args)
    return _combine(parts)


kernel._last_results = None
